# revision 8
# baseline (speedup 1.0000x reference)
"""Trainium2 Bass kernel for nn_ConvM_Layer (episode covariance similarity).

Math reformulation (exact):
  cov      = S_c S_c^T / (hw-1)  with S_c the per-(t,way) centered support (c x 500)
  cov_sim  = q^T cov q = ||S_c^T q||^2 / (hw-1)  >= 0   (PSD quadratic form)
  => LeakyReLU is the identity, and
  score[t,q,w] = sum_p conv_w[p]/(hw-1) * ||S_c^T (q_p - qbar)||^2 + conv_b

v2 layout: query spatial columns are packed 128-per-PE-tile (full output
rows, 30 tiles x 5 ways x 5 k = 750 matmuls of 500 columns), operands in
bf16 (halves DMA, keeps 1 cyc/row streams, hides weight loads), row-wise
square-accumulate split across Scalar/Vector/GpSimd engines, and the
final conv_w-weighted segment sum done with one PSUM-accumulated matmul
chain against a host-built segment matrix.

Sharding: 8 cores = (t in 0..3) x (wq half in 0..1); wq padded 75 -> 76 = 2*38.
"""

from contextlib import ExitStack

import ml_dtypes
import numpy as np

import concourse.bass as bass
import concourse.tile as tile
from concourse import bacc, mybir
from concourse.bass_utils import run_bass_kernel_spmd

# Problem shape (hardcoded per contract)
T, WQ, C, H, W = 4, 75, 640, 10, 10
HW = H * W                 # 100
WAY, SHOT = 5, 5
M = SHOT * HW              # 500 support samples per way
WQP = 76                   # padded query count (divisible by 2)
WQH = WQP // 2             # 38 queries per core
NQ = WQH * HW              # 3800 query spatial columns per core
NQP = 3840                 # padded to 30 * 128
NT = NQP // 128            # 30 query-row PE tiles
CT = C // 128              # 5 contraction tiles
N_CORES = 8
QCH = 5                    # queries per DMA/centering chunk (500 cols)
N_WARM = 36                # dummy matmuls that pre-warm the PE clock gate

F32 = mybir.dt.float32
BF16 = mybir.dt.bfloat16

_CACHE: dict = {}


def _qchunks():
    out = []
    c0 = 0
    while c0 < NQ:
        out.append((c0, min(QCH * HW, NQ - c0)))
        c0 += QCH * HW
    return out


def _kernel_body(ctx: ExitStack, tc: tile.TileContext, q_d, s_d, w_d, o_d):
    nc = tc.nc
    X = mybir.AxisListType.X

    sraw_p = ctx.enter_context(tc.tile_pool(name="sraw", bufs=3))
    sc_p = ctx.enter_context(tc.tile_pool(name="sc", bufs=WAY * CT))
    qraw_p = ctx.enter_context(tc.tile_pool(name="qraw", bufs=3))
    qc_p = ctx.enter_context(tc.tile_pool(name="qc", bufs=1))
    stat_p = ctx.enter_context(tc.tile_pool(name="stat", bufs=6))
    tr_s_p = ctx.enter_context(tc.tile_pool(name="trs", bufs=2))
    tr_v_p = ctx.enter_context(tc.tile_pool(name="trv", bufs=2))
    lcs_p = ctx.enter_context(tc.tile_pool(name="lcs", bufs=1))
    w_p = ctx.enter_context(tc.tile_pool(name="wgt", bufs=1))
    osb_p = ctx.enter_context(tc.tile_pool(name="osb", bufs=1))
    warm_p = ctx.enter_context(tc.tile_pool(name="warm", bufs=1))
    ps_p = ctx.enter_context(tc.tile_pool(name="ps", bufs=WAY, space="PSUM"))
    wps_p = ctx.enter_context(tc.tile_pool(name="wps", bufs=1, space="PSUM"))
    ops_p = ctx.enter_context(tc.tile_pool(name="ops", bufs=1, space="PSUM"))

    # ---- PE warm-up: dependency-free matmuls on a zeroed tile ----
    wsrc = warm_p.tile([128, 512], BF16, name="wsrc")
    nc.vector.memset(wsrc[:], 0.0)
    wps = wps_p.tile([128, 512], F32, name="wpsum")
    for _ in range(N_WARM):
        nc.tensor.matmul(wps[:], wsrc[:, :128], wsrc[:], start=True, stop=True)

    # segment matrix [128, NT*WQH]
    w_sb = w_p.tile([128, NT * WQH], F32)
    nc.sync.dma_start(w_sb[:], w_d[:])

    # persistent centered-query tiles, filled chunk by chunk
    q_c = [
        qc_p.tile([128, NQP], BF16, name=f"qc{ct}", tag=f"qc{ct}")
        for ct in range(CT)
    ]
    s_c = [[None] * CT for _ in range(WAY)]

    def load_center_support(wy):
        for ct in range(CT):
            sraw = sraw_p.tile([128, M], BF16)
            nc.sync.dma_start(
                sraw[:], s_d[ct * 128:(ct + 1) * 128, wy * M:(wy + 1) * M]
            )
            smean = stat_p.tile([128, 1], F32, tag="smean")
            nc.vector.reduce_sum(smean[:], sraw[:], axis=X)
            nc.vector.tensor_scalar_mul(smean[:], smean[:], 1.0 / M)
            sc = sc_p.tile([128, M], BF16)
            nc.gpsimd.tensor_scalar_sub(sc[:], sraw[:], smean[:])
            s_c[wy][ct] = sc

    def load_center_query(c0, ncols):
        nq = ncols // HW
        cols = slice(c0, c0 + ncols)
        for ct in range(CT):
            qraw = qraw_p.tile([128, QCH * HW], BF16)
            nc.sync.dma_start(
                qraw[:, :ncols], q_d[ct * 128:(ct + 1) * 128, cols]
            )
            qsum = stat_p.tile([128, QCH], F32, tag="qsum")
            nc.vector.reduce_sum(
                qsum[:, :nq],
                qraw[:, :ncols].rearrange("c (q h) -> c q h", h=HW),
                axis=X,
            )
            nc.vector.tensor_scalar_mul(qsum[:, :nq], qsum[:, :nq], 1.0 / HW)
            nc.vector.tensor_sub(
                q_c[ct][:, cols].rearrange("c (q h) -> c q h", h=HW),
                qraw[:, :ncols].rearrange("c (q h) -> c q h", h=HW),
                qsum[:, :nq].broadcast_to((128, nq, HW)),
            )

    # support way 0 and the first query chunk first so the PE can start early
    load_center_support(0)
    chunks = _qchunks()
    load_center_query(*chunks[0])
    for wy in range(1, WAY):
        load_center_support(wy)
    for ct in range(CT):
        nc.vector.memset(q_c[ct][:, NQ:NQP], 0.0)
    for c0, ncols in chunks[1:]:
        load_center_query(c0, ncols)

    # ---- main: P = Q_tile^T S_c per (qtile, way); lcs col = rowwise ||.||^2 ----
    lcs = lcs_p.tile([128, NT * WAY], F32)
    for ti in range(NT):
        qslice = q_c_slice = slice(ti * 128, (ti + 1) * 128)
        for wy in range(WAY):
            ps = ps_p.tile([128, M], F32)
            for ct in range(CT):
                nc.tensor.matmul(
                    ps[:],
                    q_c[ct][:, qslice],
                    s_c[wy][ct][:],
                    start=(ct == 0),
                    stop=(ct == CT - 1),
                )
            col = ti * WAY + wy
            # GpSimd cannot read PSUM and the DVE may read only one PSUM
            # operand, so: Scalar squares straight from PSUM; Vector first
            # stages the tile to SBUF as bf16 (one PSUM read), then
            # square-reduces at the 16-bit DVE rate.
            if wy in (0, 2, 3):
                trash = tr_s_p.tile([128, M], F32)
                nc.scalar.activation(
                    trash[:], ps[:], mybir.ActivationFunctionType.Square,
                    accum_out=lcs[:, col:col + 1],
                )
            else:
                sbf = tr_v_p.tile([128, M], BF16, tag="sbf")
                nc.vector.tensor_scalar_mul(sbf[:], ps[:], 1.0)
                trash = tr_v_p.tile([128, M], BF16, tag="trv")
                nc.vector.tensor_tensor_reduce(
                    out=trash[:], in0=sbf[:], in1=sbf[:], scale=1.0, scalar=0.0,
                    op0=mybir.AluOpType.mult, op1=mybir.AluOpType.add,
                    accum_out=lcs[:, col:col + 1],
                )

    # ---- score[q, w] = sum_ti wseg_ti^T @ lcs_ti  -> [WQH, WAY] ----
    ops = ops_p.tile([WQH, WAY], F32)
    for ti in range(NT):
        nc.tensor.matmul(
            ops[:],
            w_sb[:, ti * WQH:(ti + 1) * WQH],
            lcs[:, ti * WAY:(ti + 1) * WAY],
            start=(ti == 0),
            stop=(ti == NT - 1),
        )
    osb = osb_p.tile([WQH, WAY], F32)
    nc.scalar.copy(osb[:], ops[:])
    nc.sync.dma_start(o_d[:], osb[:])


def _build():
    key = "nc"
    if key in _CACHE:
        return _CACHE[key]
    nc = bacc.Bacc(
        "TRN2", target_bir_lowering=False, debug=False, num_devices=N_CORES
    )
    q_d = nc.dram_tensor("q", [C, NQP], BF16, kind="ExternalInput").ap()
    s_d = nc.dram_tensor("s", [C, WAY * M], BF16, kind="ExternalInput").ap()
    w_d = nc.dram_tensor("w", [128, NT * WQH], F32, kind="ExternalInput").ap()
    o_d = nc.dram_tensor("out", [WQH, WAY], F32, kind="ExternalOutput").ap()
    with tile.TileContext(nc) as tc:
        with ExitStack() as ctx:
            _kernel_body(ctx, tc, q_d, s_d, w_d, o_d)
    nc.compile()
    _CACHE[key] = nc
    return nc


def _make_wseg(conv_w):
    w = (np.asarray(conv_w, dtype=np.float64)[0, 0] / (HW - 1)).astype(np.float64)
    wseg = np.zeros((128, NT * WQH), dtype=np.float32)
    for ti in range(NT):
        g = ti * 128 + np.arange(128)
        qg = g // HW
        p = g % HW
        valid = qg < WQH
        wseg[np.arange(128)[valid], ti * WQH + qg[valid]] = w[p[valid]]
    return wseg


def make_in_maps(query_feat, support_feat, conv_w):
    q = np.asarray(query_feat, dtype=np.float32).reshape(T, WQ, C, HW)
    s = np.asarray(support_feat, dtype=np.float32).reshape(T, WAY * SHOT, C, HW)
    wseg = _make_wseg(conv_w)
    # channel-major transposes so every DMA partition-row is contiguous;
    # width 2*NQ + pad so both halves can slice a full [C, NQP] window
    # (cols >= NQ per half are never read by the device: chunks stop at NQ
    # and the on-device memset zeroes q_c[:, NQ:NQP])
    qt = np.zeros((T, C, 2 * NQ + (NQP - NQ)), dtype=ml_dtypes.bfloat16)
    qt[:, :, :WQ * HW] = q.transpose(0, 2, 1, 3).reshape(T, C, WQ * HW)
    st = np.ascontiguousarray(
        s.transpose(0, 2, 1, 3).reshape(T, C, WAY * M)
    ).astype(ml_dtypes.bfloat16)
    in_maps = []
    for core in range(N_CORES):
        ti, half = core // 2, core % 2
        in_maps.append({
            "q": np.ascontiguousarray(qt[ti, :, half * NQ:half * NQ + NQP]),
            "s": st[ti],
            "w": wseg,
        })
    return in_maps


LAST_RESULT = None  # set by kernel(); lets a harness read exec_time_ns/profile


def kernel(query_feat, support_feat, conv_w, conv_b):
    global LAST_RESULT
    nc = _build()
    in_maps = make_in_maps(query_feat, support_feat, conv_w)
    res = run_bass_kernel_spmd(nc, in_maps, list(range(N_CORES)))
    LAST_RESULT = res
    score = np.empty((T, WQP, WAY), dtype=np.float32)
    for core in range(N_CORES):
        ti, half = core // 2, core % 2
        score[ti, half * WQH:(half + 1) * WQH, :] = res.results[core]["out"]
    out = score[:, :WQ, :] + np.asarray(conv_b, dtype=np.float32)[0]
    return np.ascontiguousarray(out)
